# revision 20
# baseline (speedup 1.0000x reference)
"""Trainium2 Bass kernel for Swin-style windowed cosine attention.

Problem: nn_Attention_8100308321041
  q,k,v: [512, 8, 256, 16] f32; table: [961, 8]; index: [65536] i64;
  mask: [64, 256, 256] f32; out: [512, 256, 128] f32.

Strategy (8 NeuronCores, pure data-parallel):
  - Shard window-instances b by (b % 64) % 8 == core -> 64 instances/core,
    ordered (wl, img) so each per-window bias+mask table chunk is fetched
    once and reused across 8 images.
  - Host prep: l2-normalize q/k -> bf16 4-head row-group layout (partition
    32*g + d); gather table[index] + mask into three per-window tables:
      CbA: C = bias+mask additive, bf16 (pair0 identity-preload path)
      CbE: exp(C), bf16            (ACT+mult path)
      Cp:  A*C + B, fp32           (Schraudolph STT path)
    v_aug carries a ones column: AV emits numerators AND softmax
    denominators; the final divide happens on HOST (raw av dump shipped
    back as bf16).
  - Device per instance, three exp paths balanced across engines (the PE
    runs at the 1.2 GHz cold p-state throughout -- measured):
      ID_ACT:   identity-preload C into PSUM (PE) + ScalarE exp
      MULT_ACT: ScalarE exp(S) + VectorE bf16 2x multiply by exp(C)
      STT:      VectorE Schraudolph int16 bitcast, C'-add fused (1x, PSUM)
    QK is a single 16-matmul burst in 4-way row-group concurrency; AV of
    the previous instance is emitted after it (never head-of-line blocks).
"""

import os
import sys

sys.path.insert(0, "/opt/trn_rl_repo")

import numpy as np
import ml_dtypes

import concourse.bass as bass
import concourse.bacc as bacc
import concourse.mybir as mybir
from concourse import tile
from concourse.bass_utils import run_bass_kernel_spmd

BF16 = ml_dtypes.bfloat16

B_, H, N, D = 512, 8, 256, 16
NW = 64          # windows per image
M_CORES = 8
IMG = B_ // NW   # 8 images
WL = NW // M_CORES  # 8 distinct windows per core
NI = IMG * WL    # 64 instances per core
HD = H * D       # 128
EPS = 1e-12
CBA_WL = 2 * 2 * N     # additive-C cols per wl: pair0 heads 0,1   (1024)
CBE_WL = 6 * 2 * N     # exp(C) cols per wl: heads 0,1,2,3,6,7    (3072)
CP_WL = 4 * 2 * N      # Schraudolph C' cols per wl: heads 4..7   (2048)
A16 = 128.0 / float(np.log(2.0))     # Schraudolph scale for bf16-via-int16
B16 = 127.0 * 128.0 - 5.09           # Schraudolph bias (round-to-nearest c)

# per-pair exp paths, by inst parity (pair0) / inst%4 (pair3)
ID_ACT, MULT_ACT, STT = 0, 1, 2
# head index within CbE table (slot order: h0,h1,h2,h3,h6,h7)
CBE_SLOT = {0: 0, 1: 1, 2: 2, 3: 3, 6: 4, 7: 5}

_NC_CACHE = {}


def _paths(inst):
    return [
        ID_ACT if inst % 2 == 0 else MULT_ACT,
        MULT_ACT,
        STT,
        MULT_ACT if inst % 2 == 1 else STT,
    ]


def build_bass(trace_sim=False):
    nc = bacc.Bacc("TRN2", target_bir_lowering=False, debug=False, num_devices=M_CORES)
    qk8 = nc.declare_dram_parameter("qk8", [NI, 128, 2 * 2 * N], mybir.dt.bfloat16, isOutput=False)
    vA = nc.declare_dram_parameter("vA", [NI, 128, 2 * H * 17], mybir.dt.bfloat16, isOutput=False)
    CbA = nc.declare_dram_parameter("CbA", [128, WL * CBA_WL], mybir.dt.bfloat16, isOutput=False)
    CbE = nc.declare_dram_parameter("CbE", [128, WL * CBE_WL], mybir.dt.bfloat16, isOutput=False)
    Cp = nc.declare_dram_parameter("Cp", [128, WL * CP_WL], mybir.dt.float32, isOutput=False)
    Ib = nc.declare_dram_parameter("Ib", [128, 128], mybir.dt.bfloat16, isOutput=False)
    out = nc.declare_dram_parameter("out", [NI, 128, 2 * H * 17], mybir.dt.bfloat16, isOutput=True)

    FP32 = mybir.dt.float32
    BF = mybir.dt.bfloat16
    I16 = mybir.dt.int16
    Exp = mybir.ActivationFunctionType.Exp
    Copy = mybir.ActivationFunctionType.Copy

    with tile.TileContext(nc, trace_sim=trace_sim) as tc:
        with (
            tc.tile_pool(name="const", bufs=1) as constp,
            tc.tile_pool(name="qk", bufs=4) as qkp,
            tc.tile_pool(name="vp", bufs=4) as vp,
            tc.tile_pool(name="pp", bufs=10) as ppool,
            tc.tile_pool(name="p0", bufs=4) as p0pool,
            tc.tile_pool(name="op", bufs=3) as opool,
            tc.tile_pool(name="ps", bufs=4, space=bass.MemorySpace.PSUM) as psp,
        ):
            catile = constp.tile([128, WL * CBA_WL], BF)
            cetile = constp.tile([128, WL * CBE_WL], BF)
            cptile = constp.tile([128, WL * CP_WL], FP32)
            itile = constp.tile([128, 128], BF)
            nc.gpsimd.dma_start(itile[:], Ib[:])

            def fetch_c(wl):
                nc.gpsimd.dma_start(catile[:, wl * CBA_WL:(wl + 1) * CBA_WL], CbA[:, wl * CBA_WL:(wl + 1) * CBA_WL])
                nc.gpsimd.dma_start(cetile[:, wl * CBE_WL:(wl + 1) * CBE_WL], CbE[:, wl * CBE_WL:(wl + 1) * CBE_WL])
                nc.gpsimd.dma_start(cptile[:, wl * CP_WL:(wl + 1) * CP_WL], Cp[:, wl * CP_WL:(wl + 1) * CP_WL])

            fetch_c(0)
            fetch_c(1)

            pending = []  # deferred AV + out work from the previous instance

            def emit_av(p_state, pairs):
                (p_inst, p_vtile, p_pb, avps) = p_state
                for pr in pairs:
                    pbf = p_pb[pr]
                    for hh in range(2):
                        h = 2 * pr + hh
                        hoff = hh * 512
                        for nck in range(2):
                            for mc in range(2):
                                nc.tensor.matmul(
                                    avps[:, nck * (H * 17) + h * 17: nck * (H * 17) + h * 17 + 17],
                                    pbf[:, hoff + mc * 256 + nck * 128: hoff + mc * 256 + nck * 128 + 128],
                                    p_vtile[:, mc * (H * 17) + h * 17: mc * (H * 17) + h * 17 + 17],
                                    start=(mc == 0), stop=(mc == 1),
                                )

            def emit_out(p_state):
                # bf16 copy of numerators+denominators; host divides.
                # Alternate engines to balance Scalar/Vector load.
                (p_inst, p_vtile, p_pb, avps) = p_state
                otile = opool.tile([128, 2 * H * 17], BF, name="otile")
                if p_inst % 2 == 0:
                    nc.vector.tensor_copy(otile[:], avps)
                else:
                    nc.scalar.activation(otile[:], avps, Copy)
                nc.gpsimd.dma_start(out[p_inst], otile[:])

            def fetch_inst(i):
                qt = qkp.tile([128, 2 * 2 * N], BF, name="qktile")
                vt = vp.tile([128, 2 * H * 17], BF, name="vtile")
                qk_eng = nc.sync if (i % 4) != 3 else nc.gpsimd
                qk_eng.dma_start(qt[:], qk8[i])
                nc.gpsimd.dma_start(vt[:], vA[i])
                return qt, vt

            inst_tiles = {0: fetch_inst(0)}

            for inst in range(NI):
                wl = inst // IMG
                if inst % IMG == 0 and wl + 2 < WL:
                    fetch_c(wl + 2)
                qktile, vtile = inst_tiles.pop(inst)
                if inst + 1 < NI:
                    inst_tiles[inst + 1] = fetch_inst(inst + 1)
                qk5 = qktile[:].rearrange("p (s q n) -> p s q n", s=2, q=2)

                paths = _paths(inst)

                pstiles = []
                for pr in range(4):
                    ps = psp.tile([128, 1024], FP32, name="ps", tag="ps")
                    pstiles.append(ps)
                avps_full = psp.tile([128, 1024], FP32, name="avps", tag="ps")

                # Phase 1: identity C-preloads (pair0 on even insts only)
                for pr in range(4):
                    if paths[pr] == ID_ACT:
                        coff = wl * CBA_WL + (2 * pr) * 2 * N
                        for hh in range(2):
                            nc.tensor.matmul(
                                pstiles[pr][:, hh * 512:(hh + 1) * 512],
                                itile[:], catile[:, coff + hh * 512: coff + (hh + 1) * 512],
                                start=True, stop=False, skip_group_check=True,
                            )

                def qk_burst(half):
                    for mc in range(2):
                        for g in range(4):
                            h = 4 * half + g
                            pr = h // 2
                            hoff = (h % 2) * 512
                            qkh = qk5[32 * g: 32 * g + D, half]
                            nc.tensor.matmul(
                                pstiles[pr][:, hoff + mc * 256: hoff + mc * 256 + 256],
                                qkh[:, 1, mc * 128:(mc + 1) * 128],
                                qkh[:, 0, :],
                                start=(paths[pr] != ID_ACT) and mc == 0,
                                stop=(mc == 1),
                                skip_group_check=True,
                                tile_position=(32 * g, 0),
                            )

                # Phase 2/3: QK half0, then previous instance's AV + out-copy
                # (before QK half1 so the pool rotation can never deadlock),
                # then QK half1
                qk_burst(0)
                if pending:
                    emit_av(pending[0], (0, 1))
                qk_burst(1)
                if pending:
                    emit_av(pending[0], (2, 3))
                    emit_out(pending[0])
                    pending.clear()

                # Phase 4: exp per pair (STT first on DVE: gated on QK only)
                ptiles = [None] * 4
                for pr in range(4):
                    if paths[pr] == STT:
                        ptile = ppool.tile([128, 1024], I16, name="pt", tag="pt")
                        poff = wl * CP_WL + (2 * pr - 4) * 512
                        nc.vector.scalar_tensor_tensor(
                            ptile[:], pstiles[pr][:], A16, cptile[:, poff: poff + 1024],
                            mybir.AluOpType.mult, mybir.AluOpType.add,
                        )
                        ptiles[pr] = ptile[:].bitcast(BF)
                for pr in range(4):
                    if paths[pr] == ID_ACT:
                        ptile = ppool.tile([128, 1024], BF, name="pt", tag="pt")
                        nc.scalar.activation(ptile[:], pstiles[pr][:], Exp)
                        ptiles[pr] = ptile[:]
                    elif paths[pr] == MULT_ACT:
                        p0tile = p0pool.tile([128, 1024], BF, name="p0t", tag="p0t")
                        nc.scalar.activation(p0tile[:], pstiles[pr][:], Exp)
                        ptile = ppool.tile([128, 1024], BF, name="pt", tag="pt")
                        s0 = CBE_SLOT[2 * pr]
                        eoff = wl * CBE_WL + s0 * 512
                        nc.vector.tensor_mul(
                            ptile[:], p0tile[:], cetile[:, eoff: eoff + 1024],
                        )
                        ptiles[pr] = ptile[:]

                pending.append((inst, vtile, ptiles, avps_full[:, 0:2 * H * 17]))

            if pending:
                emit_av(pending[0], (0, 1, 2, 3))
                emit_out(pending[0])
                pending.clear()
    nc.compile()
    return nc


def _host_prep(q, k, v, table, index, mask):
    """Returns per-core input maps + the inverse b-index map."""
    qn = q / np.maximum(np.sqrt((q * q).sum(-1, keepdims=True)), EPS)
    kn = k / np.maximum(np.sqrt((k * k).sum(-1, keepdims=True)), EPS)
    # 4-head row-group layout: [b, g, d(padded to 32), half, qk, n], h = 4*half+g
    qk8 = np.zeros((B_, 4, 32, 2, 2, N), np.float32)
    qk8[:, :, :D, :, 0] = qn.transpose(0, 1, 3, 2).reshape(B_, 2, 4, D, N).transpose(0, 2, 3, 1, 4)
    qk8[:, :, :D, :, 1] = kn.transpose(0, 1, 3, 2).reshape(B_, 2, 4, D, N).transpose(0, 2, 3, 1, 4)
    qk8 = qk8.reshape(B_, 128, 2 * 2 * N).astype(BF16)
    # v_aug [b, n, h, 17] -> [b, mc, 128, h, 17] -> [b, 128, mc*h*17]
    vA = np.empty((B_, N, H, 17), np.float32)
    vA[..., :16] = v.transpose(0, 2, 1, 3)
    vA[..., 16] = 1.0
    vA = vA.reshape(B_, 2, 128, H * 17).transpose(0, 2, 1, 3).reshape(B_, 128, 2 * H * 17).astype(BF16)
    # bias'[h, m, n] = table[index[n*256+m], h]
    bias = table[index.astype(np.int64)].reshape(N, N, H).transpose(2, 1, 0)  # [h, m, n]
    maskT = mask.transpose(0, 2, 1)  # [w, m, n]

    in_maps = []
    b_order = []
    ident = np.eye(128, dtype=BF16)
    for c in range(M_CORES):
        bs = np.array([img * NW + (c + M_CORES * wl) for wl in range(WL) for img in range(IMG)])
        b_order.append(bs)
        C = (bias[None, :, :, :] + maskT[c::M_CORES][:, None, :, :]).astype(np.float32)
        C = C.reshape(WL, H, 2, 128, N)
        # pair0 additive path: heads 0,1 bf16
        CbA_ = C[:, :2].transpose(3, 0, 1, 2, 4).reshape(128, WL * CBA_WL).astype(BF16)
        # exp(C) path: heads 0,1,2,3,6,7 bf16
        CbE_ = np.exp(C[:, [0, 1, 2, 3, 6, 7]]).transpose(3, 0, 1, 2, 4).reshape(128, WL * CBE_WL).astype(BF16)
        # Schraudolph path: heads 4..7 fp32 pre-scaled A*C + B
        Cp_ = (A16 * C[:, 4:] + B16).transpose(3, 0, 1, 2, 4).reshape(128, WL * CP_WL).astype(np.float32)
        in_maps.append({
            "qk8": np.ascontiguousarray(qk8[bs]),
            "vA": np.ascontiguousarray(vA[bs]),
            "CbA": CbA_,
            "CbE": CbE_,
            "Cp": Cp_,
            "Ib": ident,
        })
    return in_maps, b_order


def kernel(q, k, v, table, index, mask):
    q = np.asarray(q, np.float32)
    k = np.asarray(k, np.float32)
    v = np.asarray(v, np.float32)
    table = np.asarray(table, np.float32)
    index = np.asarray(index)
    mask = np.asarray(mask, np.float32)

    in_maps, b_order = _host_prep(q, k, v, table, index, mask)

    if "nc" not in _NC_CACHE:
        _NC_CACHE["nc"] = build_bass()
    nc = _NC_CACHE["nc"]

    res = run_bass_kernel_spmd(nc, in_maps, core_ids=list(range(M_CORES)))
    out = np.empty((B_, N, HD), np.float32)
    for c in range(M_CORES):
        # av dump [NI, 128, (nck h x)] bf16: x = 16 numerators + denominator
        arr = res.results[c]["out"].astype(np.float32).reshape(NI, 128, 2, H, 17)
        o = arr[..., :16] / arr[..., 16:17]           # [NI, p, nck, H, D]
        out[b_order[c]] = o.transpose(0, 2, 1, 3, 4).reshape(NI, N, HD)
    return out


if __name__ == "__main__":
    rng = np.random.default_rng(0)
    q = rng.standard_normal((B_, H, N, D), dtype=np.float32)
    k = rng.standard_normal((B_, H, N, D), dtype=np.float32)
    v = rng.standard_normal((B_, H, N, D), dtype=np.float32)
    table = rng.standard_normal((961, H), dtype=np.float32)
    index = rng.integers(0, 961, size=(N * N,)).astype(np.int64)
    mask = rng.standard_normal((NW, N, N), dtype=np.float32)
    o = kernel(q=q, k=k, v=v, table=table, index=index, mask=mask)
    print("out", o.shape, o.dtype, float(np.abs(o).mean()))
